# revision 7
# baseline (speedup 1.0000x reference)
"""MinimalRNN Trainium2 kernel.

Data-parallel over batch: B=64 -> 8 per core, weights replicated.

Per core (B_loc=8, T=512, I=512, H=1024):
  P0: load Wz/W/V, PE-transpose them into SBUF (WzT, WT, VT; fp32).
  P1: per batch element c (8 chunks of 512 tokens):
        xT  = PE-transpose(x[c])                     (i on partitions)
        zT  = tanh(WzT.T @ xT + bz)   (psum -> ACT)  kept in SBUF (zc)
        zn  = PE-transpose(zT) -> DRAM (z natural, (t,h) rows)
        zv  = zT.T @ VT + b (ones-row fold) -> DRAM  (tokens, H)
  P2: recurrence over t:
        pre(8,1024) = accum_k  hT[:,k8].T @ WT[k]  +  I8.T @ zv_t   (PSUM)
        u = sigmoid(pre)  (ACT, psum->sbuf)
        h = zn_t + u*(h_prev - zn_t)      (DVE, natural (8,1024) layout)
        hT = PE-transpose(h) (8 cheap shots) -> next step's stationary
        DMA h -> out[:, t, :]
All matmuls run as float32r (FP22 truncated multiply, fp32 accumulate),
1 PE cycle/row at N>=256.
"""

import numpy as np
import concourse.bass as bass
import concourse.bacc as bacc
import concourse.mybir as mybir
import concourse.tile as tile
from concourse.masks import make_identity
from concourse.bass_utils import run_bass_kernel_spmd

B, T, I, H = 64, 512, 512, 1024
N_CORES = 8
BL = B // N_CORES          # 8 batch elements per core
KT = H // 128              # 8  h k-tiles
IT = I // 128              # 4  i k-tiles
TS = T // 128              # 4  token subtiles per chunk
F32 = mybir.dt.float32
F32R = mybir.dt.float32r
AF = mybir.ActivationFunctionType


def _r(ap):
    return ap.bitcast(F32R)


def _build_program():
    nc = bacc.Bacc("TRN2", target_bir_lowering=False, debug=False)
    x_d = nc.dram_tensor("x", (BL, T, I), F32, kind="ExternalInput")
    wz_d = nc.dram_tensor("Wz", (H, I), F32, kind="ExternalInput")
    w_d = nc.dram_tensor("W", (H, H), F32, kind="ExternalInput")
    v_d = nc.dram_tensor("V", (H, H), F32, kind="ExternalInput")
    bz_d = nc.dram_tensor("bz", (H,), F32, kind="ExternalInput")
    b_d = nc.dram_tensor("b", (H,), F32, kind="ExternalInput")
    out_d = nc.dram_tensor("out", (BL, T, H), F32, kind="ExternalOutput")
    hn_d = nc.dram_tensor("hn", (BL, H), F32, kind="ExternalOutput")
    zn_d = nc.dram_tensor("zn", (BL, T, H), F32, kind="Internal")
    zv_d = nc.dram_tensor("zv", (BL, T, H), F32R, kind="Internal")

    with tile.TileContext(nc) as tc:
        _kernel(tc, x_d, wz_d, w_d, v_d, bz_d, b_d, out_d, hn_d, zn_d, zv_d)
    nc.compile()
    return nc


def _kernel(tc, x_d, wz_d, w_d, v_d, bz_d, b_d, out_d, hn_d, zn_d, zv_d):
    nc = tc.nc

    # ---------- persistent constants ----------
    consts_cm = tc.tile_pool(name="consts", bufs=1)
    consts = consts_cm.__enter__()
    ident = consts.tile([128, 128], F32)
    make_identity(nc, ident)
    eye8t = consts.tile([8, 8], F32)
    make_identity(nc, eye8t)
    eye8 = consts.tile([8, 8], F32R)
    nc.vector.tensor_copy(eye8[:], eye8t[:])
    ones1f = consts.tile([1, 128], F32)
    nc.gpsimd.memset(ones1f[:], 1.0)
    ones1 = consts.tile([1, 128], F32R)
    nc.vector.tensor_copy(ones1[:], ones1f[:])
    bz_sb = consts.tile([128, KT], F32)     # bz_sb[p, m] = bz[m*128+p]
    nc.sync.dma_start(bz_sb[:], bz_d.rearrange("(m p) -> p m", p=128))
    b_raw = consts.tile([1, H], F32)
    nc.sync.dma_start(b_raw[:], b_d[None, :])
    b_sb = consts.tile([1, H], F32R)
    nc.vector.tensor_copy(b_sb[:], b_raw[:])
    # transposed weights, all [k-part, cols]
    wt = consts.tile([128, KT, H], F32R)      # WT[p,k,g] = W[g, k*128+p]
    vt = consts.tile([128, KT, H], F32R)      # VT[p,k,g] = V[g, k*128+p]
    wzt = consts.tile([128, IT, H], F32R)     # WzT[p,it,h] = Wz[h, it*128+p]

    # ---------- P0: transpose weights ----------
    with (
        tc.tile_pool(name="p0_load", bufs=2) as p0l,
        tc.tile_pool(name="p0_psum", bufs=4, space="PSUM") as p0p,
    ):
        for (src, dst, ncols, nrows) in (
            (w_d, wt, KT, KT),    # natural (H rows, H cols)
            (v_d, vt, KT, KT),
            (wz_d, wzt, IT, KT),  # natural (H=1024 rows, I=512 cols)
        ):
            n_out = nrows  # row-tiles of the natural matrix = col blocks of dst
            for gt in range(n_out):
                nat = p0l.tile([128, ncols * 128], F32, tag="p0nat")
                nc.sync.dma_start(nat[:], src[gt * 128:(gt + 1) * 128, :])
                for kt in range(ncols):
                    ps = p0p.tile([128, 128], F32, tag="p0ps")
                    nc.tensor.transpose(ps[:], nat[:, kt * 128:(kt + 1) * 128], ident[:])
                    nc.vector.tensor_copy(dst[:, kt, gt * 128:(gt + 1) * 128], ps[:])

    # ---------- P1: z = tanh(x@WzT+bz), zn, zv = z@V.T + b ----------
    with (
        tc.tile_pool(name="p1_xn", bufs=2) as pxn,
        tc.tile_pool(name="p1_xt", bufs=2) as pxt,
        tc.tile_pool(name="p1_zc", bufs=2) as pzc,
        tc.tile_pool(name="p1_zn", bufs=2) as pzn,
        tc.tile_pool(name="p1_zv", bufs=3) as pzv,
        tc.tile_pool(name="p1_ps1", bufs=2, space="PSUM") as pps1,
        tc.tile_pool(name="p1_ps2", bufs=2, space="PSUM") as pps2,
        tc.tile_pool(name="p1_pst", bufs=2, space="PSUM") as ppst,
    ):
        for c in range(BL):
            xn = pxn.tile([128, TS, I], F32, tag="xn")
            nc.sync.dma_start(
                xn[:], x_d[c].rearrange("(tt p) i -> p tt i", p=128))
            xt = pxt.tile([128, IT, T], F32R, tag="xt")
            for it in range(IT):
                for tt in range(TS):
                    ps = ppst.tile([128, 128], F32, tag="pst")
                    nc.tensor.transpose(
                        ps[:], xn[:, tt, it * 128:(it + 1) * 128], ident[:])
                    nc.vector.tensor_copy(
                        xt[:, it, tt * 128:(tt + 1) * 128], ps[:])
            # GEMM1: zc[p, m, t] = tanh(bz + sum_i Wz.T)
            zc = pzc.tile([128, KT, T], F32R, tag="zc")
            zn_sb = pzn.tile([128, TS, H], F32, tag="znsb")
            for m in range(KT):
                ps1 = pps1.tile([128, T], F32, tag="ps1")
                for it in range(IT):
                    nc.tensor.matmul(
                        ps1[:],
                        wzt[:, it, m * 128:(m + 1) * 128],
                        xt[:, it, :],
                        start=(it == 0), stop=(it == IT - 1))
                nc.scalar.activation(
                    zc[:, m, :], ps1[:], AF.Tanh, bias=bz_sb[:, m:m + 1])
                # z natural: transpose back (t on partitions)
                for tt in range(TS):
                    ps = ppst.tile([128, 128], F32, tag="pst")
                    nc.tensor.transpose(
                        ps[:], zc[:, m, tt * 128:(tt + 1) * 128].bitcast(F32),
                        ident[:])
                    nc.vector.tensor_copy(
                        zn_sb[:, tt, m * 128:(m + 1) * 128], ps[:])
            nc.sync.dma_start(
                zn_d[c].rearrange("(tt p) h -> p tt h", p=128), zn_sb[:])
            # GEMM2: zv[tok, g] = z @ V.T + b
            for s in range(TS):
                for g in range(2):
                    ps2 = pps2.tile([128, 512], F32, tag="ps2")
                    for k in range(KT):
                        nc.tensor.matmul(
                            ps2[:],
                            zc[:, k, s * 128:(s + 1) * 128],
                            vt[:, k, g * 512:(g + 1) * 512],
                            start=(k == 0), stop=False)
                    nc.tensor.matmul(
                        ps2[:], ones1[:, 0:128],
                        b_sb[:, g * 512:(g + 1) * 512],
                        start=False, stop=True)
                    zvs = pzv.tile([128, 512], F32R, tag="zvs")
                    nc.vector.tensor_copy(zvs[:], ps2[:])
                    nc.sync.dma_start(
                        zv_d[c, s * 128:(s + 1) * 128, g * 512:(g + 1) * 512],
                        zvs[:])

    # ---------- P2: recurrence ----------
    with (
        tc.tile_pool(name="p2_zv", bufs=4) as p2zv,
        tc.tile_pool(name="p2_zn", bufs=4) as p2zn,
        tc.tile_pool(name="p2_u", bufs=4) as p2u,
        tc.tile_pool(name="p2_d", bufs=4) as p2d,
        tc.tile_pool(name="p2_h", bufs=3) as p2h,
        tc.tile_pool(name="p2_ht", bufs=3) as p2ht,
        tc.tile_pool(name="p2_pre", bufs=4, space="PSUM") as p2pre,
        tc.tile_pool(name="p2_pst", bufs=2, space="PSUM") as p2pst,
    ):
        h_prev = p2h.tile([8, H], F32, tag="h")
        nc.gpsimd.memset(h_prev[:], 0.0)
        htz = p2ht.tile([128, KT * 8], F32, tag="htz")
        nc.gpsimd.memset(htz[:], 0.0)
        ht_prev = p2ht.tile([128, KT * 8], F32R, tag="ht")
        nc.vector.tensor_copy(ht_prev[:], htz[:])

        for t in range(T):
            zvt = p2zv.tile([8, H], F32R, tag="zvt")
            nc.sync.dma_start(zvt[:], zv_d[:, t, :])
            znt = p2zn.tile([8, H], F32, tag="znt")
            nc.sync.dma_start(znt[:], zn_d[:, t, :])

            h_new = p2h.tile([8, H], F32, tag="h")
            ht_new = p2ht.tile([128, KT * 8], F32R, tag="ht")
            ht_ps = p2pst.tile([128, KT * 8], F32, tag="htps")

            for g in range(2):
                cols = slice(g * 512, (g + 1) * 512)
                pre = p2pre.tile([8, 512], F32, tag="pre")
                for k in range(KT):
                    nc.tensor.matmul(
                        pre[:],
                        ht_prev[:, k * 8:(k + 1) * 8],
                        wt[:, k, cols],
                        start=(k == 0), stop=False)
                nc.tensor.matmul(
                    pre[:], eye8[:], zvt[:, cols],
                    start=False, stop=True)
                u_g = p2u.tile([8, 512], F32, tag="u")
                nc.scalar.activation(u_g[:], pre[:], AF.Sigmoid)
                # natural-land update: h = zn + u * (h_prev - zn)
                d_g = p2d.tile([8, 512], F32, tag="d")
                nc.vector.tensor_sub(d_g[:], h_prev[:, cols], znt[:, cols])
                nc.vector.tensor_mul(d_g[:], u_g[:], d_g[:])
                nc.vector.tensor_add(h_new[:, cols], znt[:, cols], d_g[:])
                # transpose h half into stationary layout
                for kk in range(4):
                    k = g * 4 + kk
                    nc.tensor.transpose(
                        ht_ps[:, k * 8:(k + 1) * 8],
                        h_new[:, k * 128:(k + 1) * 128], eye8t[:])
            nc.vector.tensor_copy(ht_new[:], ht_ps[:])

            nc.sync.dma_start(out_d[:, t, :], h_new[:])
            if t == T - 1:
                nc.sync.dma_start(hn_d[:], h_new[:])
            h_prev, ht_prev = h_new, ht_new

    consts_cm.__exit__(None, None, None)


_CACHE = {}


def _get_program():
    if "nc" not in _CACHE:
        _CACHE["nc"] = _build_program()
    return _CACHE["nc"]


def kernel(x, Wz, bz, W, V, b):
    nc = _get_program()
    x = np.ascontiguousarray(x, dtype=np.float32)
    shared = {
        "Wz": np.ascontiguousarray(Wz, dtype=np.float32),
        "W": np.ascontiguousarray(W, dtype=np.float32),
        "V": np.ascontiguousarray(V, dtype=np.float32),
        "bz": np.ascontiguousarray(bz, dtype=np.float32),
        "b": np.ascontiguousarray(b, dtype=np.float32),
    }
    in_maps = [
        {"x": x[c * BL:(c + 1) * BL], **shared} for c in range(N_CORES)
    ]
    res = run_bass_kernel_spmd(nc, in_maps, core_ids=list(range(N_CORES)))
    outputs = np.concatenate([res.results[c]["out"] for c in range(N_CORES)], axis=0)
    h_n = np.concatenate([res.results[c]["hn"] for c in range(N_CORES)], axis=0)
    return outputs, h_n
